# revision 4
# baseline (speedup 1.0000x reference)
"""AnchorDeformAtt (deformable attention) on 8 TRN2 NeuronCores.

Sharding: core m -> batch b = m//4, head pair (2*(m%4), 2*(m%4)+1).
Per core:
  - fused 1x1-conv projections (value/size/anchor/att) as PE matmuls
  - per-point bilinear taps resolved via a per-head "4-tap table" in DRAM
    (row r = [P[r], P[r+1], P[r+W], P[r+W+1]], bf16, 256B rows) gathered
    with gpsimd dma_gather (one 256B row per sample point)
  - tap/point reduction: DVE multiplies + adds, then the 16-point sum is
    done on the TensorEngine as 16 accumulating matmuls against identity
    (which also transposes head_out into [head_dim, l] layout)
  - AllToAll over each 4-core batch group swaps head-shards for l-shards,
    then each core computes out_proj + BN for its quarter of L.
Host assembles the 8 (C, L/4) quarters into the full output.
"""
import sys

sys.path.insert(0, '/opt/trn_rl_repo')

import numpy as np

B, C, H, W = 2, 256, 64, 96
L = H * W              # 6144
NH, NP, HD = 8, 16, 32
NT = L // 128          # 48 l-tiles
LQ = L // 8            # 768, per-core output columns (per batch)
NPROJ = 164            # fused projection output columns
PCOLS = 100            # staged non-value projection columns
EPS = 1e-6
TWO23 = 8388608.0

_CACHE = {}


def _build_nc():
    import concourse.mybir as mybir
    import concourse.tile as tile
    from concourse import bacc
    from concourse.masks import make_identity

    fp32 = mybir.dt.float32
    bf16 = mybir.dt.bfloat16
    i16 = mybir.dt.int16
    A = mybir.AluOpType
    AF = mybir.ActivationFunctionType

    nc = bacc.Bacc("TRN2", target_bir_lowering=False, num_devices=8)

    feat = nc.dram_tensor("feat", [C, L], fp32, kind="ExternalInput")
    wproj = nc.dram_tensor("wproj", [C, NPROJ], fp32, kind="ExternalInput")
    bproj = nc.dram_tensor("bproj", [1, NPROJ], fp32, kind="ExternalInput")
    wot = nc.dram_tensor("wot", [C, C], fp32, kind="ExternalInput")
    bnsc = nc.dram_tensor("bnsc", [128, 2], fp32, kind="ExternalInput")
    bnbi = nc.dram_tensor("bnbi", [128, 2], fp32, kind="ExternalInput")
    cent = nc.dram_tensor("cent", [128, 96], fp32, kind="ExternalInput")
    out = nc.dram_tensor("out", [2 * C, LQ], fp32, kind="ExternalOutput")

    with tile.TileContext(nc) as tc:
        with (
            tc.tile_pool(name="const", bufs=1) as cpool,
            tc.tile_pool(name="pers", bufs=1) as ppool,
            tc.tile_pool(name="work", bufs=3) as wpool,
            tc.tile_pool(name="tmp", bufs=1) as tpool,
            tc.tile_pool(name="psA", bufs=2, space="PSUM") as pspool,
            tc.tile_pool(name="psT", bufs=2, space="PSUM") as psT,
            tc.tile_pool(name="psO", bufs=2, space="PSUM") as psO,
            tc.tile_pool(name="dram", bufs=1, space="DRAM") as dpool,
        ):
            # ---- constants ----
            wproj_sb = cpool.tile([128, 2, NPROJ], fp32)
            nc.sync.dma_start(
                wproj_sb[:], wproj[:, :].rearrange("(cc p) n -> p cc n", cc=2))
            bias_sb = cpool.tile([1, NPROJ], fp32)
            nc.sync.dma_start(bias_sb[:], bproj[:, :])
            wot_sb = cpool.tile([128, 2, C], fp32)
            nc.sync.dma_start(
                wot_sb[:], wot[:, :].rearrange("(kc p) n -> p kc n", kc=2))
            bnsc_sb = cpool.tile([128, 2], fp32)
            nc.sync.dma_start(bnsc_sb[:], bnsc[:, :])
            bnbi_sb = cpool.tile([128, 2], fp32)
            nc.sync.dma_start(bnbi_sb[:], bnbi[:, :])
            cent_sb = cpool.tile([128, 96], fp32)
            nc.sync.dma_start(cent_sb[:], cent[:, :])
            ones1 = cpool.tile([1, 128], fp32)
            nc.vector.memset(ones1[:], 1.0)
            ident = cpool.tile([128, 128], fp32)
            make_identity(nc, ident[:])

            # ---- persistent ----
            P_sb = ppool.tile([128, 49, 64], bf16)    # value, l=t*128+p rows
            nc.vector.memset(P_sb[:], 0.0)
            proj_sb = ppool.tile([128, NT, PCOLS], fp32)
            C4 = [ppool.tile([128, NT * 64], bf16, tag=f"c4_{h}", name=f"c4_{h}") for h in (0, 1)]
            Rr = [ppool.tile([128, NT * 16], i16, tag=f"r_{h}", name=f"r_{h}") for h in (0, 1)]
            IX = [ppool.tile([128, NT * 128], i16, tag=f"ix_{h}", name=f"ix_{h}") for h in (0, 1)]
            HO = ppool.tile([64, NT, 128], fp32)
            T_dram = [dpool.tile([L, 128], bf16, tag=f"tab_{h}", name=f"tab_{h}") for h in (0, 1)]
            ho_bounce = dpool.tile([512, LQ], fp32)
            a2a_out = dpool.tile([512, LQ], fp32)

            # ---- phase B: fused projections ----
            for t in range(NT):
                ps = pspool.tile([128, NPROJ], fp32)
                for cc in range(2):
                    ft = wpool.tile([128, 128], fp32, tag="ft")
                    nc.sync.dma_start(
                        ft[:],
                        feat[cc * 128:(cc + 1) * 128, t * 128:(t + 1) * 128])
                    nc.tensor.matmul(ps[:], ft[:], wproj_sb[:, cc, :],
                                     start=(cc == 0), stop=False)
                nc.tensor.matmul(ps[:], ones1[:], bias_sb[:],
                                 start=False, stop=True)
                nc.scalar.activation(P_sb[:, t, :], ps[:, 0:64], AF.Copy)
                nc.scalar.activation(proj_sb[:, t, :], ps[:, 64:NPROJ], AF.Copy)

            # ---- phase C: nonlinearities + weights + indices ----
            nc.scalar.activation(proj_sb[:, :, 0:68], proj_sb[:, :, 0:68],
                                 AF.Sigmoid)
            nc.vector.tensor_scalar(out=proj_sb[:, :, 0:4],
                                    in0=proj_sb[:, :, 0:4],
                                    scalar1=0.25, scalar2=0.75,
                                    op0=A.max, op1=A.min)
            nc.scalar.activation(proj_sb[:, :, 68:100], proj_sb[:, :, 68:100],
                                 AF.Exp)

            shp = [128, NT, 16]
            for h in (0, 1):
                sx = proj_sb[:, :, 2 * h:2 * h + 1]        # [128,48,1]
                sy = proj_sb[:, :, 2 * h + 1:2 * h + 2]
                anc = proj_sb[:, :, 4 + 32 * h:4 + 32 * h + 32].rearrange(
                    "q t (p j) -> q t p j", j=2)
                ox, oy = anc[:, :, :, 0], anc[:, :, :, 1]
                att = proj_sb[:, :, 68 + 16 * h:68 + 16 * h + 16]
                cx, cy = cent_sb[:, 0:48], cent_sb[:, 48:96]

                axc = tpool.tile([128, NT], fp32, tag="axc")
                nc.vector.scalar_tensor_tensor(
                    out=axc[:], in0=sx[:, :, 0], scalar=-0.5, in1=cx,
                    op0=A.mult, op1=A.add)
                ayc = tpool.tile([128, NT], fp32, tag="ayc")
                nc.vector.scalar_tensor_tensor(
                    out=ayc[:], in0=sy[:, :, 0], scalar=-0.5, in1=cy,
                    op0=A.mult, op1=A.add)

                def floorpath(o_ap, s_ap, a_t, scale, tagp):
                    # returns (frac, floor) tiles [128, NT, 16]
                    tp = tpool.tile(shp, fp32, tag=f"tp{tagp}")
                    tr = tpool.tile(shp, fp32, tag=f"tr{tagp}")
                    tg = tpool.tile(shp, fp32, tag="tg", name=f"tg{tagp}")
                    nc.vector.tensor_tensor(
                        out=tp[:], in0=o_ap, in1=s_ap.to_broadcast(shp),
                        op=A.mult)
                    nc.vector.tensor_tensor(
                        out=tp[:], in0=tp[:],
                        in1=a_t[:][:, :, None].to_broadcast(shp), op=A.add)
                    nc.vector.tensor_scalar(out=tp[:], in0=tp[:],
                                            scalar1=0.0, scalar2=1.0,
                                            op0=A.max, op1=A.min)
                    nc.vector.tensor_scalar(out=tr[:], in0=tp[:],
                                            scalar1=scale, scalar2=TWO23,
                                            op0=A.mult, op1=A.add)
                    nc.vector.tensor_scalar(out=tr[:], in0=tr[:],
                                            scalar1=TWO23, scalar2=None,
                                            op0=A.subtract)
                    nc.vector.tensor_scalar(out=tp[:], in0=tp[:],
                                            scalar1=scale, scalar2=None,
                                            op0=A.mult)
                    nc.vector.tensor_tensor(out=tg[:], in0=tr[:], in1=tp[:],
                                            op=A.is_gt)
                    nc.vector.tensor_tensor(out=tr[:], in0=tr[:], in1=tg[:],
                                            op=A.subtract)     # floor
                    nc.vector.tensor_tensor(out=tp[:], in0=tp[:], in1=tr[:],
                                            op=A.subtract)     # frac
                    return tp, tr

                wx, x0 = floorpath(ox, sx, axc, float(W - 1), "x")
                wy, y0 = floorpath(oy, sy, ayc, float(H - 1), "y")

                rf = tpool.tile(shp, fp32, tag="tg", name="rf")
                nc.vector.scalar_tensor_tensor(
                    out=rf[:], in0=y0[:], scalar=float(W), in1=x0[:],
                    op0=A.mult, op1=A.add)
                nc.vector.tensor_scalar(
                    out=Rr[h][:].rearrange("q (t p) -> q t p", p=16),
                    in0=rf[:], scalar1=0.0, scalar2=None, op0=A.add)

                ex = tpool.tile(shp, fp32, tag="ex")
                nc.vector.tensor_scalar(out=ex[:], in0=wx[:], scalar1=-1.0,
                                        scalar2=1.0, op0=A.mult, op1=A.add)
                ey = tpool.tile(shp, fp32, tag="ey")
                nc.vector.tensor_scalar(out=ey[:], in0=wy[:], scalar1=-1.0,
                                        scalar2=1.0, op0=A.mult, op1=A.add)

                asum = tpool.tile([128, NT], fp32, tag="asum")
                nc.vector.tensor_reduce(out=asum[:], in_=att,
                                        axis=mybir.AxisListType.X, op=A.add)
                arec = tpool.tile([128, NT], fp32, tag="arec")
                nc.vector.reciprocal(arec[:], asum[:])
                an = tpool.tile(shp, fp32, tag="an")
                nc.vector.tensor_tensor(
                    out=an[:], in0=att,
                    in1=arec[:][:, :, None].to_broadcast(shp), op=A.mult)
                m0 = tpool.tile(shp, fp32, tag="m0")
                nc.vector.tensor_tensor(out=m0[:], in0=an[:], in1=ey[:],
                                        op=A.mult)
                nc.vector.tensor_tensor(out=an[:], in0=an[:], in1=wy[:],
                                        op=A.mult)             # an = m1
                c4v = C4[h][:].rearrange("q (t p s) -> q t p s", p=16, s=4)
                nc.vector.tensor_tensor(out=c4v[:, :, :, 0], in0=m0[:],
                                        in1=ex[:], op=A.mult)
                nc.vector.tensor_tensor(out=c4v[:, :, :, 1], in0=m0[:],
                                        in1=wx[:], op=A.mult)
                nc.vector.tensor_tensor(out=c4v[:, :, :, 2], in0=an[:],
                                        in1=ex[:], op=A.mult)
                nc.vector.tensor_tensor(out=c4v[:, :, :, 3], in0=an[:],
                                        in1=wx[:], op=A.mult)

                # index rearrange into gather layout + 8x replication
                for qh in range(8):
                    nc.sync.dma_start(
                        IX[h][0:16, :].rearrange(
                            "ql (t p e) -> ql t p e", p=16, e=8)[:, :, :, qh],
                        Rr[h][qh * 16:(qh + 1) * 16, :].rearrange(
                            "ql (t p) -> ql t p", p=16))
                nc.sync.dma_start(IX[h][16:32, :], IX[h][0:16, :])
                nc.sync.dma_start(IX[h][32:64, :], IX[h][0:32, :])
                nc.sync.dma_start(IX[h][64:128, :], IX[h][0:64, :])

            # ---- phase D: 4-tap tables in DRAM ----
            for h in (0, 1):
                tv = T_dram[h][:, :].rearrange("(t p) e -> p t e", p=128)
                for blk, sh in enumerate((0, 1, W, W + 1)):
                    pa = 128 - sh
                    nc.sync.dma_start(
                        tv[0:pa, :, blk * 32:(blk + 1) * 32],
                        P_sb[sh:128, 0:48, 32 * h:32 * h + 32])
                    if sh:
                        nc.sync.dma_start(
                            tv[pa:128, :, blk * 32:(blk + 1) * 32],
                            P_sb[0:sh, 1:49, 32 * h:32 * h + 32])

            # ---- phase E: gather + combine + point-reduce ----
            for b2 in range(NT // 2):
                gs = []
                for h in (0, 1):
                    G = wpool.tile([128, 32, 128], mybir.dt.bfloat16, tag="G")
                    nc.gpsimd.dma_gather(
                        out_ap=G[:],
                        in_ap=T_dram[h][:, :],
                        idxs_ap=IX[h][:, b2 * 256:(b2 + 1) * 256],
                        num_idxs=4096,
                        num_idxs_reg=4096,
                        elem_size=128,
                        single_packet=False,
                    )
                    gs.append(G)
                S1 = wpool.tile([128, 2, 16, 64], fp32, tag="S1", bufs=2)
                for h in (0, 1):
                    G = gs[h]
                    gv = G[:].rearrange("q b (t d) -> q b t d", t=4)
                    c4b = C4[h][:, b2 * 128:(b2 + 1) * 128].rearrange(
                        "q (b t) -> q b t", t=4)[:, :, :, None].to_broadcast(
                            [128, 32, 4, 32])
                    nc.vector.tensor_tensor(out=gv, in0=gv, in1=c4b, op=A.mult)
                    U = wpool.tile([128, 32, 32], bf16, tag="U", bufs=2)
                    V = wpool.tile([128, 32, 32], bf16, tag="V", bufs=2)
                    nc.vector.tensor_tensor(out=U[:], in0=G[:, :, 0:32],
                                            in1=G[:, :, 32:64], op=A.add)
                    nc.vector.tensor_tensor(out=V[:], in0=G[:, :, 64:96],
                                            in1=G[:, :, 96:128], op=A.add)
                    nc.vector.tensor_tensor(
                        out=S1[:, :, :, h * 32:(h + 1) * 32],
                        in0=U[:].rearrange("q (lt p) d -> q lt p d", lt=2),
                        in1=V[:].rearrange("q (lt p) d -> q lt p d", lt=2),
                        op=A.add)
                for lt in (0, 1):
                    t = 2 * b2 + lt
                    pst = psT.tile([64, 128], fp32)
                    for p in range(16):
                        nc.tensor.matmul(pst[:], S1[:, lt, p, :], ident[:],
                                         start=(p == 0), stop=(p == 15))
                    nc.scalar.activation(HO[:, t, :], pst[:], AF.Copy)

            # ---- phase F: 8-core AllToAll + out_proj + BN ----
            # core m sends its (batch, head-pair) ho slice for l-slice j to
            # core j; each core ends with ALL (b, h) channels for its L/8.
            for j in range(8):
                nc.sync.dma_start(
                    ho_bounce[j * 64:(j + 1) * 64, :].rearrange(
                        "r (t e) -> r t e", e=128),
                    HO[:, j * 6:(j + 1) * 6, :])
            nc.gpsimd.collective_compute(
                "AllToAll",
                A.bypass,
                replica_groups=[[0, 1, 2, 3, 4, 5, 6, 7]],
                ins=[ho_bounce.opt()],
                outs=[a2a_out.opt()],
            )
            # a2a_out rows: (bb, kc, p) -> channel kc*128+p of batch bb
            rhs_sb = ppool.tile([128, 4, LQ], fp32)
            nc.sync.dma_start(
                rhs_sb[:],
                a2a_out[:, :].rearrange("(bb kc p) n -> p (bb kc) n",
                                        bb=2, kc=2))
            out_sb = ppool.tile([128, 4, LQ], fp32)
            for bb in range(2):
                for cc in range(2):
                    for l0, ln in ((0, 512), (512, 256)):
                        pso = psO.tile([128, 512], fp32)
                        for kc in range(2):
                            nc.tensor.matmul(
                                pso[:, 0:ln],
                                wot_sb[:, kc, cc * 128:(cc + 1) * 128],
                                rhs_sb[:, 2 * bb + kc, l0:l0 + ln],
                                start=(kc == 0), stop=(kc == 1))
                        nc.vector.tensor_scalar(
                            out=out_sb[:, 2 * bb + cc, l0:l0 + ln],
                            in0=pso[:, 0:ln],
                            scalar1=bnsc_sb[:, cc:cc + 1],
                            scalar2=bnbi_sb[:, cc:cc + 1],
                            op0=A.mult, op1=A.add)
            nc.sync.dma_start(
                out[:, :].rearrange("(q p) n -> p q n", q=4), out_sb[:])

    nc.finalize()
    return nc


def _prep_inputs(inputs):
    f = np.float32
    feat_sd = np.asarray(inputs['feat_sd'], dtype=f)
    w_size = np.asarray(inputs['w_size'], dtype=f)
    b_size = np.asarray(inputs['b_size'], dtype=f)
    w_anchor = np.asarray(inputs['w_anchor'], dtype=f)
    b_anchor = np.asarray(inputs['b_anchor'], dtype=f)
    w_value = np.asarray(inputs['w_value'], dtype=f)
    b_value = np.asarray(inputs['b_value'], dtype=f)
    w_att = np.asarray(inputs['w_att'], dtype=f)
    b_att = np.asarray(inputs['b_att'], dtype=f)
    w_out = np.asarray(inputs['w_out'], dtype=f)
    bn_gamma = np.asarray(inputs['bn_gamma'], dtype=f)
    bn_beta = np.asarray(inputs['bn_beta'], dtype=f)
    bn_mean = np.asarray(inputs['bn_mean'], dtype=f)
    bn_var = np.asarray(inputs['bn_var'], dtype=f)

    wot = np.ascontiguousarray(w_out.T)
    scale = (bn_gamma / np.sqrt(bn_var + np.float32(1e-5))).astype(f)
    bias = (bn_beta - bn_mean * scale).astype(f)
    bnsc = np.ascontiguousarray(scale.reshape(2, 128).T)
    bnbi = np.ascontiguousarray(bias.reshape(2, 128).T)

    l = np.arange(L).reshape(NT, 128)
    cx = ((l % W + 0.5).astype(f) / np.float32(W + EPS)).T
    cy = ((l // W + 0.5).astype(f) / np.float32(H + EPS)).T
    cent = np.ascontiguousarray(np.concatenate([cx, cy], axis=1), dtype=f)

    in_maps = []
    for m in range(8):
        b = m // 4
        h0 = 2 * (m % 4)
        h1 = h0 + 1
        wrows = np.concatenate([
            w_value[h0 * 32:(h0 + 1) * 32],
            w_value[h1 * 32:(h1 + 1) * 32],
            w_size[[2 * h0, 2 * h0 + 1, 2 * h1, 2 * h1 + 1]],
            w_anchor[h0 * 32:(h0 + 1) * 32],
            w_anchor[h1 * 32:(h1 + 1) * 32],
            w_att[h0 * 16:(h0 + 1) * 16],
            w_att[h1 * 16:(h1 + 1) * 16],
        ], axis=0)
        brows = np.concatenate([
            b_value[h0 * 32:(h0 + 1) * 32],
            b_value[h1 * 32:(h1 + 1) * 32],
            b_size[[2 * h0, 2 * h0 + 1, 2 * h1, 2 * h1 + 1]],
            b_anchor[h0 * 32:(h0 + 1) * 32],
            b_anchor[h1 * 32:(h1 + 1) * 32],
            b_att[h0 * 16:(h0 + 1) * 16],
            b_att[h1 * 16:(h1 + 1) * 16],
        ], axis=0)
        in_maps.append({
            "feat": np.ascontiguousarray(feat_sd[b].reshape(C, L)),
            "wproj": np.ascontiguousarray(wrows.T),
            "bproj": np.ascontiguousarray(brows.reshape(1, NPROJ)),
            "wot": wot,
            "bnsc": bnsc,
            "bnbi": bnbi,
            "cent": cent,
        })
    return in_maps


def _run(inputs, trace=False):
    from concourse.bass_utils import run_bass_kernel_spmd
    if "nc" not in _CACHE:
        _CACHE["nc"] = _build_nc()
    nc = _CACHE["nc"]
    in_maps = _prep_inputs(inputs)
    res = run_bass_kernel_spmd(nc, in_maps, core_ids=list(range(8)),
                               trace=trace)
    full = np.empty((B, C, L), np.float32)
    for m in range(8):
        o = res.results[m]["out"].reshape(2, C, LQ)
        for bb in range(2):
            full[bb][:, m * LQ:(m + 1) * LQ] = o[bb]
    return full.reshape(B, C, H, W), res.exec_time_ns


def kernel(**inputs):
    out, _ = _run(inputs, trace=False)
    return out


# revision 11
# speedup vs baseline: 1.6974x; 1.6974x over previous
"""AnchorDeformAtt (deformable attention) on 8 TRN2 NeuronCores.

Sharding: core m -> batch b = m//4, head pair (2*(m%4), 2*(m%4)+1).
Per core:
  - fused 1x1-conv projections (value/size/anchor/att) as PE matmuls
  - per-point bilinear taps resolved via a per-head "4-tap table" in DRAM
    (row r = [P[r], P[r+1], P[r+W], P[r+W+1]], bf16, 256B rows) gathered
    with gpsimd dma_gather (one 256B row per sample point)
  - tap/point reduction: DVE multiplies + adds, then the 16-point sum is
    done on the TensorEngine as 16 accumulating matmuls against identity
    (which also transposes head_out into [head_dim, l] layout)
  - AllToAll over each 4-core batch group swaps head-shards for l-shards,
    then each core computes out_proj + BN for its quarter of L.
Host assembles the 8 (C, L/4) quarters into the full output.
"""
import sys

sys.path.insert(0, '/opt/trn_rl_repo')

import numpy as np

B, C, H, W = 2, 256, 64, 96
L = H * W              # 6144
NH, NP, HD = 8, 16, 32
NT = L // 128          # 48 l-tiles
LQ = L // 8            # 768, per-core output columns (per batch)
NPROJ = 164            # fused projection output columns
PCOLS = 100            # staged non-value projection columns
EPS = 1e-6
TWO23 = 8388608.0

_CACHE = {}


def _build_nc():
    import concourse.mybir as mybir
    import concourse.tile as tile
    from concourse import bacc
    from concourse.masks import make_identity

    fp32 = mybir.dt.float32
    bf16 = mybir.dt.bfloat16
    i16 = mybir.dt.int16
    A = mybir.AluOpType
    AF = mybir.ActivationFunctionType

    nc = bacc.Bacc("TRN2", target_bir_lowering=False, num_devices=8,
                   num_swdge_queues=4)

    feat = nc.dram_tensor("feat", [C, L], fp32, kind="ExternalInput")
    wproj = nc.dram_tensor("wproj", [C, NPROJ], fp32, kind="ExternalInput")
    bproj = nc.dram_tensor("bproj", [1, NPROJ], fp32, kind="ExternalInput")
    wot = nc.dram_tensor("wot", [C, C], bf16, kind="ExternalInput")
    bnsc = nc.dram_tensor("bnsc", [128, 2], fp32, kind="ExternalInput")
    bnbi = nc.dram_tensor("bnbi", [128, 2], fp32, kind="ExternalInput")
    cent = nc.dram_tensor("cent", [128, 96], fp32, kind="ExternalInput")
    out = nc.dram_tensor("out", [2 * C, LQ], fp32, kind="ExternalOutput")

    with tile.TileContext(nc) as tc:
        with (
            tc.tile_pool(name="const", bufs=1) as cpool,
            tc.tile_pool(name="pers", bufs=1) as ppool,
            tc.tile_pool(name="work", bufs=3) as wpool,
            tc.tile_pool(name="tmp", bufs=1) as tpool,
            tc.tile_pool(name="psA", bufs=2, space="PSUM") as pspool,
            tc.tile_pool(name="psT", bufs=2, space="PSUM") as psT,
            tc.tile_pool(name="psO", bufs=2, space="PSUM") as psO,
            tc.tile_pool(name="dram", bufs=1, space="DRAM") as dpool,
        ):
            # ---- constants ----
            wproj_sb = cpool.tile([128, 2, NPROJ], fp32)
            nc.sync.dma_start(
                wproj_sb[:], wproj[:, :].rearrange("(cc p) n -> p cc n", cc=2))
            bias_sb = cpool.tile([1, NPROJ], fp32)
            nc.sync.dma_start(bias_sb[:], bproj[:, :])
            wot_sb = cpool.tile([128, 2, C], bf16)
            nc.sync.dma_start(
                wot_sb[:], wot[:, :].rearrange("(kc p) n -> p kc n", kc=2))
            bnsc_sb = cpool.tile([128, 2], fp32)
            nc.sync.dma_start(bnsc_sb[:], bnsc[:, :])
            bnbi_sb = cpool.tile([128, 2], fp32)
            nc.sync.dma_start(bnbi_sb[:], bnbi[:, :])
            cent_sb = cpool.tile([128, 96], fp32)
            nc.sync.dma_start(cent_sb[:], cent[:, :])
            ones1 = cpool.tile([1, 128], fp32)
            nc.vector.memset(ones1[:], 1.0)
            ident = cpool.tile([128, 128], bf16)
            make_identity(nc, ident[:])
            shmats = {}
            for sh in (1, W, W + 1):
                sa = cpool.tile([128, 128], bf16, tag=f"sha{sh}", name=f"sha{sh}")
                nc.gpsimd.memset(sa[:], 0.0)
                nc.gpsimd.affine_select(
                    out=sa[:], in_=sa[:], compare_op=A.not_equal, fill=1.0,
                    base=-sh, pattern=[[-1, 128]], channel_multiplier=1)
                sb_ = cpool.tile([128, 128], bf16, tag=f"shb{sh}", name=f"shb{sh}")
                nc.gpsimd.memset(sb_[:], 0.0)
                nc.gpsimd.affine_select(
                    out=sb_[:], in_=sb_[:], compare_op=A.not_equal, fill=1.0,
                    base=128 - sh, pattern=[[-1, 128]], channel_multiplier=1)
                shmats[sh] = (sa, sb_)

            # ---- persistent ----
            P_sb = ppool.tile([128, 49, 64], bf16)    # value, l=t*128+p rows
            nc.vector.memset(P_sb[:], 0.0)
            proj_sb = ppool.tile([128, NT, PCOLS], fp32, tag="bigb", name="proj_sb",
                                 padded_shape=[128, NT, PCOLS])
            C4 = [ppool.tile([128, NT * 64], bf16, tag=f"c4_{h}", name=f"c4_{h}") for h in (0, 1)]
            Rr = [ppool.tile([128, NT * 16], i16, tag=f"r_{h}", name=f"r_{h}") for h in (0, 1)]
            IX = [ppool.tile([128, NT * 128], i16, tag=f"ix_{h}", name=f"ix_{h}") for h in (0, 1)]
            HO = ppool.tile([64, NT, 128], bf16)
            T_sb = ppool.tile([128, 2, NT, 128], bf16)
            T_dram = [dpool.tile([L, 128], bf16, tag=f"tab_{h}", name=f"tab_{h}") for h in (0, 1)]
            ho_bounce = dpool.tile([512, LQ], bf16)
            a2a_out = dpool.tile([512, LQ], bf16)

            # ---- phase B: fused projections ----
            for t in range(NT):
                ps = pspool.tile([128, NPROJ], fp32)
                for cc in range(2):
                    ft = wpool.tile([128, 128], fp32, tag="ft")
                    nc.sync.dma_start(
                        ft[:],
                        feat[cc * 128:(cc + 1) * 128, t * 128:(t + 1) * 128])
                    nc.tensor.matmul(ps[:], ft[:], wproj_sb[:, cc, :],
                                     start=(cc == 0), stop=False)
                nc.tensor.matmul(ps[:], ones1[:], bias_sb[:],
                                 start=False, stop=True)
                nc.scalar.activation(P_sb[:, t, :], ps[:, 0:64], AF.Copy)
                nc.scalar.activation(
                    T_sb[:, :, t, 0:32],
                    ps[:, 0:64].rearrange("q (h e) -> q h e", h=2), AF.Copy)
                nc.scalar.activation(proj_sb[:, t, :], ps[:, 64:NPROJ], AF.Copy)

            # ---- phase C: nonlinearities + weights + indices ----
            nc.scalar.activation(proj_sb[:, :, 0:68], proj_sb[:, :, 0:68],
                                 AF.Sigmoid)
            nc.vector.tensor_scalar(out=proj_sb[:, :, 0:4],
                                    in0=proj_sb[:, :, 0:4],
                                    scalar1=0.25, scalar2=0.75,
                                    op0=A.max, op1=A.min)
            nc.scalar.activation(proj_sb[:, :, 68:100], proj_sb[:, :, 68:100],
                                 AF.Exp)

            shp = [128, NT, 16]
            for h in (0, 1):
                sx = proj_sb[:, :, 2 * h:2 * h + 1]        # [128,48,1]
                sy = proj_sb[:, :, 2 * h + 1:2 * h + 2]
                anc = proj_sb[:, :, 4 + 32 * h:4 + 32 * h + 32].rearrange(
                    "q t (p j) -> q t p j", j=2)
                ox, oy = anc[:, :, :, 0], anc[:, :, :, 1]
                att = proj_sb[:, :, 68 + 16 * h:68 + 16 * h + 16]
                cx, cy = cent_sb[:, 0:48], cent_sb[:, 48:96]

                axc = tpool.tile([128, NT], fp32, tag="axc")
                nc.vector.scalar_tensor_tensor(
                    out=axc[:], in0=sx[:, :, 0], scalar=-0.5, in1=cx,
                    op0=A.mult, op1=A.add)
                ayc = tpool.tile([128, NT], fp32, tag="ayc")
                nc.vector.scalar_tensor_tensor(
                    out=ayc[:], in0=sy[:, :, 0], scalar=-0.5, in1=cy,
                    op0=A.mult, op1=A.add)

                def floorpath(o_ap, s_ap, a_t, scale, tagp):
                    # returns (frac, floor) tiles [128, NT, 16]
                    tp = tpool.tile(shp, fp32, tag=f"tp{tagp}")
                    tr = tpool.tile(shp, fp32, tag=f"tr{tagp}")
                    tg = tpool.tile(shp, fp32, tag="tg", name=f"tg{tagp}")
                    nc.vector.tensor_tensor(
                        out=tp[:], in0=o_ap, in1=s_ap.to_broadcast(shp),
                        op=A.mult)
                    nc.vector.tensor_tensor(
                        out=tp[:], in0=tp[:],
                        in1=a_t[:][:, :, None].to_broadcast(shp), op=A.add)
                    nc.vector.tensor_scalar(out=tp[:], in0=tp[:],
                                            scalar1=0.0, scalar2=1.0,
                                            op0=A.max, op1=A.min)
                    nc.vector.tensor_scalar(out=tr[:], in0=tp[:],
                                            scalar1=scale, scalar2=TWO23,
                                            op0=A.mult, op1=A.add)
                    nc.vector.tensor_scalar(out=tr[:], in0=tr[:],
                                            scalar1=TWO23, scalar2=None,
                                            op0=A.subtract)
                    nc.vector.tensor_scalar(out=tp[:], in0=tp[:],
                                            scalar1=scale, scalar2=None,
                                            op0=A.mult)
                    nc.vector.tensor_tensor(out=tg[:], in0=tr[:], in1=tp[:],
                                            op=A.is_gt)
                    nc.vector.tensor_tensor(out=tr[:], in0=tr[:], in1=tg[:],
                                            op=A.subtract)     # floor
                    nc.vector.tensor_tensor(out=tp[:], in0=tp[:], in1=tr[:],
                                            op=A.subtract)     # frac
                    return tp, tr

                wx, x0 = floorpath(ox, sx, axc, float(W - 1), "x")
                wy, y0 = floorpath(oy, sy, ayc, float(H - 1), "y")

                rf = tpool.tile(shp, fp32, tag="tg", name="rf")
                nc.vector.scalar_tensor_tensor(
                    out=rf[:], in0=y0[:], scalar=float(W), in1=x0[:],
                    op0=A.mult, op1=A.add)
                # p-major table row: r' = (r % 128) * NT + r // 128
                qq = tpool.tile(shp, fp32, tag="qq", name="qq")
                gg = tpool.tile(shp, fp32, tag="gg2", name="gg2")
                nc.vector.tensor_scalar(out=qq[:], in0=rf[:],
                                        scalar1=1.0 / 128.0, scalar2=TWO23,
                                        op0=A.mult, op1=A.add)
                nc.vector.tensor_scalar(out=qq[:], in0=qq[:], scalar1=TWO23,
                                        scalar2=None, op0=A.subtract)
                nc.vector.tensor_scalar(out=gg[:], in0=rf[:],
                                        scalar1=1.0 / 128.0, scalar2=None,
                                        op0=A.mult)
                nc.vector.tensor_tensor(out=gg[:], in0=qq[:], in1=gg[:],
                                        op=A.is_gt)
                nc.vector.tensor_tensor(out=qq[:], in0=qq[:], in1=gg[:],
                                        op=A.subtract)          # r // 128
                nc.vector.scalar_tensor_tensor(
                    out=rf[:], in0=qq[:], scalar=-128.0, in1=rf[:],
                    op0=A.mult, op1=A.add)                      # r % 128
                nc.vector.scalar_tensor_tensor(
                    out=rf[:], in0=rf[:], scalar=float(NT), in1=qq[:],
                    op0=A.mult, op1=A.add)                      # r'
                nc.vector.tensor_scalar(
                    out=Rr[h][:].rearrange("q (t p) -> q t p", p=16),
                    in0=rf[:], scalar1=0.0, scalar2=None, op0=A.add)

                ex = tpool.tile(shp, fp32, tag="ex")
                nc.vector.tensor_scalar(out=ex[:], in0=wx[:], scalar1=-1.0,
                                        scalar2=1.0, op0=A.mult, op1=A.add)
                ey = tpool.tile(shp, fp32, tag="ey")
                nc.vector.tensor_scalar(out=ey[:], in0=wy[:], scalar1=-1.0,
                                        scalar2=1.0, op0=A.mult, op1=A.add)

                asum = tpool.tile([128, NT], fp32, tag="asum")
                nc.vector.tensor_reduce(out=asum[:], in_=att,
                                        axis=mybir.AxisListType.X, op=A.add)
                arec = tpool.tile([128, NT], fp32, tag="arec")
                nc.vector.reciprocal(arec[:], asum[:])
                an = tpool.tile(shp, fp32, tag="an")
                nc.vector.tensor_tensor(
                    out=an[:], in0=att,
                    in1=arec[:][:, :, None].to_broadcast(shp), op=A.mult)
                m0 = tpool.tile(shp, fp32, tag="m0")
                nc.vector.tensor_tensor(out=m0[:], in0=an[:], in1=ey[:],
                                        op=A.mult)
                nc.vector.tensor_tensor(out=an[:], in0=an[:], in1=wy[:],
                                        op=A.mult)             # an = m1
                c4v = C4[h][:].rearrange("q (t p s) -> q t p s", p=16, s=4)
                nc.vector.tensor_tensor(out=c4v[:, :, :, 0], in0=m0[:],
                                        in1=ex[:], op=A.mult)
                nc.vector.tensor_tensor(out=c4v[:, :, :, 1], in0=m0[:],
                                        in1=wx[:], op=A.mult)
                nc.vector.tensor_tensor(out=c4v[:, :, :, 2], in0=an[:],
                                        in1=ex[:], op=A.mult)
                nc.vector.tensor_tensor(out=c4v[:, :, :, 3], in0=an[:],
                                        in1=wx[:], op=A.mult)

                # index rearrange into gather layout + 8x replication
                for qh in range(8):
                    nc.sync.dma_start(
                        IX[h][0:16, :].rearrange(
                            "ql (t p e) -> ql t p e", p=16, e=8)[:, :, :, qh],
                        Rr[h][qh * 16:(qh + 1) * 16, :].rearrange(
                            "ql (t p) -> ql t p", p=16))
                nc.sync.dma_start(IX[h][16:32, :], IX[h][0:16, :])
                nc.sync.dma_start(IX[h][32:64, :], IX[h][0:32, :])
                nc.sync.dma_start(IX[h][64:128, :], IX[h][0:64, :])

            # ---- phase D: build tables on PE (partition shifts), flat DMA ----
            for t in range(NT):
                for blk, sh in enumerate((1, W, W + 1)):
                    sa, sb_ = shmats[sh]
                    psh = psT.tile([128, 64], fp32, tag="psh")
                    nc.tensor.matmul(psh[:], sa[:], P_sb[:, t, :],
                                     start=True, stop=False)
                    nc.tensor.matmul(psh[:], sb_[:], P_sb[:, t + 1, :],
                                     start=False, stop=True)
                    nc.scalar.activation(
                        T_sb[:, :, t, (blk + 1) * 32:(blk + 2) * 32],
                        psh[:].rearrange("q (h e) -> q h e", h=2), AF.Copy)
            for h in (0, 1):
                nc.sync.dma_start(
                    T_dram[h][:, :].rearrange("(p t) e -> p (t e)", p=128),
                    T_sb[:, h, :, :])

            # ---- phase E: gather + combine + point-reduce ----
            qctr = 0
            for t in range(NT):
                gs = []
                for h in (0, 1):
                    G = wpool.tile([128, 16, 128], bf16, tag="G", bufs=8)
                    nc.gpsimd.dma_gather(
                        out_ap=G[:],
                        in_ap=T_dram[h][:, :],
                        idxs_ap=IX[h][:, t * 128:(t + 1) * 128],
                        num_idxs=2048,
                        num_idxs_reg=2048,
                        elem_size=128,
                        single_packet=False,
                        queue_num=qctr % 4,
                    )
                    qctr += 1
                    gs.append(G)
                S1 = wpool.tile([128, 16, 64], bf16, tag="S1", bufs=3)
                for h in (0, 1):
                    G = gs[h]
                    gv = G[:].rearrange("q b (s d) -> q b s d", s=4)
                    c4b = C4[h][:, t * 64:(t + 1) * 64].rearrange(
                        "q (b s) -> q b s", s=4)[:, :, :, None].to_broadcast(
                            [128, 16, 4, 32])
                    nc.vector.tensor_tensor(out=gv, in0=gv, in1=c4b, op=A.mult)
                    U = wpool.tile([128, 16, 32], bf16, tag="U", bufs=3)
                    V = wpool.tile([128, 16, 32], bf16, tag="V", bufs=3)
                    nc.vector.tensor_tensor(out=U[:], in0=G[:, :, 0:32],
                                            in1=G[:, :, 32:64], op=A.add)
                    nc.vector.tensor_tensor(out=V[:], in0=G[:, :, 64:96],
                                            in1=G[:, :, 96:128], op=A.add)
                    nc.vector.tensor_tensor(
                        out=S1[:, :, h * 32:(h + 1) * 32],
                        in0=U[:], in1=V[:], op=A.add)
                pst = psT.tile([64, 128], fp32)
                for p in range(16):
                    nc.tensor.matmul(pst[:], S1[:, p, :], ident[:],
                                     start=(p == 0), stop=(p == 15))
                nc.scalar.activation(HO[:, t, :], pst[:], AF.Copy)

            # ---- phase F: 8-core AllToAll + out_proj + BN ----
            # core m sends its (batch, head-pair) ho slice for l-slice j to
            # core j; each core ends with ALL (b, h) channels for its L/8.
            for j in range(8):
                nc.sync.dma_start(
                    ho_bounce[j * 64:(j + 1) * 64, :].rearrange(
                        "r (t e) -> r t e", e=128),
                    HO[:, j * 6:(j + 1) * 6, :])
            nc.gpsimd.collective_compute(
                "AllToAll",
                A.bypass,
                replica_groups=[[0, 1, 2, 3, 4, 5, 6, 7]],
                ins=[ho_bounce.opt()],
                outs=[a2a_out.opt()],
            )
            # a2a_out rows: (bb, kc, p) -> channel kc*128+p of batch bb
            rhs_sb = ppool.tile([128, 4, LQ], bf16)
            nc.sync.dma_start(
                rhs_sb[:],
                a2a_out[:, :].rearrange("(bb kc p) n -> p (bb kc) n",
                                        bb=2, kc=2))
            out_sb = ppool.tile([128, 4, LQ], fp32, tag="bigb", name="out_sb",
                                padded_shape=[128, 4, NT * PCOLS // 4])
            for bb in range(2):
                for cc in range(2):
                    for l0, ln in ((0, 512), (512, 256)):
                        pso = psO.tile([128, 512], fp32)
                        for kc in range(2):
                            nc.tensor.matmul(
                                pso[:, 0:ln],
                                wot_sb[:, kc, cc * 128:(cc + 1) * 128],
                                rhs_sb[:, 2 * bb + kc, l0:l0 + ln],
                                start=(kc == 0), stop=(kc == 1))
                        nc.vector.tensor_scalar(
                            out=out_sb[:, 2 * bb + cc, l0:l0 + ln],
                            in0=pso[:, 0:ln],
                            scalar1=bnsc_sb[:, cc:cc + 1],
                            scalar2=bnbi_sb[:, cc:cc + 1],
                            op0=A.mult, op1=A.add)
            nc.sync.dma_start(
                out[:, :].rearrange("(q p) n -> p q n", q=4), out_sb[:])

    nc.finalize()
    return nc


def _prep_inputs(inputs):
    f = np.float32
    feat_sd = np.asarray(inputs['feat_sd'], dtype=f)
    w_size = np.asarray(inputs['w_size'], dtype=f)
    b_size = np.asarray(inputs['b_size'], dtype=f)
    w_anchor = np.asarray(inputs['w_anchor'], dtype=f)
    b_anchor = np.asarray(inputs['b_anchor'], dtype=f)
    w_value = np.asarray(inputs['w_value'], dtype=f)
    b_value = np.asarray(inputs['b_value'], dtype=f)
    w_att = np.asarray(inputs['w_att'], dtype=f)
    b_att = np.asarray(inputs['b_att'], dtype=f)
    w_out = np.asarray(inputs['w_out'], dtype=f)
    bn_gamma = np.asarray(inputs['bn_gamma'], dtype=f)
    bn_beta = np.asarray(inputs['bn_beta'], dtype=f)
    bn_mean = np.asarray(inputs['bn_mean'], dtype=f)
    bn_var = np.asarray(inputs['bn_var'], dtype=f)

    import ml_dtypes
    wot = np.ascontiguousarray(w_out.T).astype(ml_dtypes.bfloat16)
    scale = (bn_gamma / np.sqrt(bn_var + np.float32(1e-5))).astype(f)
    bias = (bn_beta - bn_mean * scale).astype(f)
    bnsc = np.ascontiguousarray(scale.reshape(2, 128).T)
    bnbi = np.ascontiguousarray(bias.reshape(2, 128).T)

    l = np.arange(L).reshape(NT, 128)
    cx = ((l % W + 0.5).astype(f) / np.float32(W + EPS)).T
    cy = ((l // W + 0.5).astype(f) / np.float32(H + EPS)).T
    cent = np.ascontiguousarray(np.concatenate([cx, cy], axis=1), dtype=f)

    in_maps = []
    for m in range(8):
        b = m // 4
        h0 = 2 * (m % 4)
        h1 = h0 + 1
        wrows = np.concatenate([
            w_value[h0 * 32:(h0 + 1) * 32],
            w_value[h1 * 32:(h1 + 1) * 32],
            w_size[[2 * h0, 2 * h0 + 1, 2 * h1, 2 * h1 + 1]],
            w_anchor[h0 * 32:(h0 + 1) * 32],
            w_anchor[h1 * 32:(h1 + 1) * 32],
            w_att[h0 * 16:(h0 + 1) * 16],
            w_att[h1 * 16:(h1 + 1) * 16],
        ], axis=0)
        brows = np.concatenate([
            b_value[h0 * 32:(h0 + 1) * 32],
            b_value[h1 * 32:(h1 + 1) * 32],
            b_size[[2 * h0, 2 * h0 + 1, 2 * h1, 2 * h1 + 1]],
            b_anchor[h0 * 32:(h0 + 1) * 32],
            b_anchor[h1 * 32:(h1 + 1) * 32],
            b_att[h0 * 16:(h0 + 1) * 16],
            b_att[h1 * 16:(h1 + 1) * 16],
        ], axis=0)
        in_maps.append({
            "feat": np.ascontiguousarray(feat_sd[b].reshape(C, L)),
            "wproj": np.ascontiguousarray(wrows.T),
            "bproj": np.ascontiguousarray(brows.reshape(1, NPROJ)),
            "wot": wot,
            "bnsc": bnsc,
            "bnbi": bnbi,
            "cent": cent,
        })
    return in_maps


def _run(inputs, trace=False):
    from concourse.bass_utils import run_bass_kernel_spmd
    if "nc" not in _CACHE:
        _CACHE["nc"] = _build_nc()
    nc = _CACHE["nc"]
    in_maps = _prep_inputs(inputs)
    res = run_bass_kernel_spmd(nc, in_maps, core_ids=list(range(8)),
                               trace=trace)
    full = np.empty((B, C, L), np.float32)
    for m in range(8):
        o = res.results[m]["out"].reshape(2, C, LQ)
        for bb in range(2):
            full[bb][:, m * LQ:(m + 1) * LQ] = o[bb]
    return full.reshape(B, C, H, W), res.exec_time_ns


def kernel(**inputs):
    out, _ = _run(inputs, trace=False)
    return out


# revision 15
# speedup vs baseline: 3.5170x; 2.0720x over previous
"""AnchorDeformAtt (deformable attention) on 8 TRN2 NeuronCores.

Sharding: core m -> batch b = m//4, head pair (2*(m%4), 2*(m%4)+1).
Per core:
  - fused 1x1-conv projections (value/size/anchor/att) as PE matmuls
  - per-point bilinear taps resolved via a per-head "4-tap table" in DRAM
    (row r = [P[r], P[r+1], P[r+W], P[r+W+1]], bf16, 256B rows) gathered
    with gpsimd dma_gather (one 256B row per sample point)
  - tap/point reduction: DVE multiplies + adds, then the 16-point sum is
    done on the TensorEngine as 16 accumulating matmuls against identity
    (which also transposes head_out into [head_dim, l] layout)
  - AllToAll over each 4-core batch group swaps head-shards for l-shards,
    then each core computes out_proj + BN for its quarter of L.
Host assembles the 8 (C, L/4) quarters into the full output.
"""
import sys

sys.path.insert(0, '/opt/trn_rl_repo')

import numpy as np

B, C, H, W = 2, 256, 64, 96
L = H * W              # 6144
NH, NP, HD = 8, 16, 32
NT = L // 128          # 48 l-tiles
LQ = L // 8            # 768, per-core output columns (per batch)
NPROJ = 164            # fused projection output columns
PCOLS = 100            # staged non-value projection columns
EPS = 1e-6
TWO23 = 8388608.0

_CACHE = {}


def _build_nc():
    import concourse.mybir as mybir
    import concourse.tile as tile
    from concourse import bacc
    from concourse.masks import make_identity

    fp32 = mybir.dt.float32
    bf16 = mybir.dt.bfloat16
    i16 = mybir.dt.int16
    A = mybir.AluOpType
    AF = mybir.ActivationFunctionType

    nc = bacc.Bacc("TRN2", target_bir_lowering=False, num_devices=8,
                   num_swdge_queues=4)

    feat = nc.dram_tensor("feat", [C, L], fp32, kind="ExternalInput")
    wproj = nc.dram_tensor("wproj", [C, NPROJ], fp32, kind="ExternalInput")
    bproj = nc.dram_tensor("bproj", [1, NPROJ], fp32, kind="ExternalInput")
    wot = nc.dram_tensor("wot", [C, C], bf16, kind="ExternalInput")
    bnsc = nc.dram_tensor("bnsc", [128, 2], fp32, kind="ExternalInput")
    bnbi = nc.dram_tensor("bnbi", [128, 2], fp32, kind="ExternalInput")
    cent = nc.dram_tensor("cent", [128, 96], fp32, kind="ExternalInput")
    fold = nc.dram_tensor("fold", [128, 24], fp32, kind="ExternalInput")
    out = nc.dram_tensor("out", [2 * C, LQ], fp32, kind="ExternalOutput")

    with tile.TileContext(nc) as tc:
        with (
            tc.tile_pool(name="const", bufs=1) as cpool,
            tc.tile_pool(name="pers", bufs=1) as ppool,
            tc.tile_pool(name="work", bufs=3) as wpool,
            tc.tile_pool(name="tmp", bufs=1) as tpool,
            tc.tile_pool(name="psA", bufs=2, space="PSUM") as pspool,
            tc.tile_pool(name="psT", bufs=2, space="PSUM") as psT,
            tc.tile_pool(name="psO", bufs=2, space="PSUM") as psO,
            tc.tile_pool(name="dram", bufs=1, space="DRAM") as dpool,
        ):
            # ---- constants ----
            wproj_sb = cpool.tile([128, 2, NPROJ], fp32)
            nc.sync.dma_start(
                wproj_sb[:], wproj[:, :].rearrange("(cc p) n -> p cc n", cc=2))
            bias_sb = cpool.tile([1, NPROJ], fp32)
            nc.sync.dma_start(bias_sb[:], bproj[:, :])
            wot_sb = cpool.tile([128, 2, C], bf16)
            nc.sync.dma_start(
                wot_sb[:], wot[:, :].rearrange("(kc p) n -> p kc n", kc=2))
            bnsc_sb = cpool.tile([128, 2], fp32)
            nc.sync.dma_start(bnsc_sb[:], bnsc[:, :])
            bnbi_sb = cpool.tile([128, 2], fp32)
            nc.sync.dma_start(bnbi_sb[:], bnbi[:, :])
            cent_sb = cpool.tile([128, 96], fp32)
            nc.sync.dma_start(cent_sb[:], cent[:, :])
            fold_sb = cpool.tile([128, 24], fp32)
            nc.sync.dma_start(fold_sb[:], fold[:, :])
            ones1 = cpool.tile([1, 128], fp32)
            nc.vector.memset(ones1[:], 1.0)
            ident = cpool.tile([128, 128], bf16)
            make_identity(nc, ident[:])
            shmats = {}
            for sh in (1, W, W + 1):
                sa = cpool.tile([128, 128], bf16, tag=f"sha{sh}", name=f"sha{sh}")
                nc.gpsimd.memset(sa[:], 0.0)
                nc.gpsimd.affine_select(
                    out=sa[:], in_=sa[:], compare_op=A.not_equal, fill=1.0,
                    base=-sh, pattern=[[-1, 128]], channel_multiplier=1)
                sb_ = cpool.tile([128, 128], bf16, tag=f"shb{sh}", name=f"shb{sh}")
                nc.gpsimd.memset(sb_[:], 0.0)
                nc.gpsimd.affine_select(
                    out=sb_[:], in_=sb_[:], compare_op=A.not_equal, fill=1.0,
                    base=128 - sh, pattern=[[-1, 128]], channel_multiplier=1)
                shmats[sh] = (sa, sb_)

            # ---- persistent ----
            P_sb = ppool.tile([128, 49, 64], bf16)    # value, l=t*128+p rows
            nc.vector.memset(P_sb[:], 0.0)
            proj_sb = ppool.tile([128, NT, PCOLS], fp32, tag="bigb", name="proj_sb",
                                 padded_shape=[128, NT, PCOLS])
            C4 = [ppool.tile([128, NT * 64], bf16, tag=f"c4_{h}", name=f"c4_{h}") for h in (0, 1)]
            Rf = [ppool.tile([128, NT * 16], fp32, tag=f"rf_{h}", name=f"rf_{h}") for h in (0, 1)]
            IX = [ppool.tile([128, NT * 128], i16, tag=f"ix_{h}", name=f"ix_{h}") for h in (0, 1)]
            HO = ppool.tile([64, NT, 128], bf16)
            T_sb = ppool.tile([128, 2, NT, 128], bf16)
            T_dram = [dpool.tile([L, 128], bf16, tag=f"tab_{h}", name=f"tab_{h}") for h in (0, 1)]
            ho_bounce = dpool.tile([512, LQ], bf16)
            a2a_out = dpool.tile([512, LQ], bf16)

            # ---- phase B: fused projections ----
            for t in range(NT):
                ps = pspool.tile([128, NPROJ], fp32)
                for cc in range(2):
                    ft = wpool.tile([128, 128], fp32, tag="ft")
                    nc.sync.dma_start(
                        ft[:],
                        feat[cc * 128:(cc + 1) * 128, t * 128:(t + 1) * 128])
                    nc.tensor.matmul(ps[:], ft[:], wproj_sb[:, cc, :],
                                     start=(cc == 0), stop=False)
                nc.tensor.matmul(ps[:], ones1[:], bias_sb[:],
                                 start=False, stop=True)
                nc.scalar.activation(P_sb[:, t, :], ps[:, 0:64], AF.Copy)
                nc.scalar.activation(
                    T_sb[:, :, t, 0:32],
                    ps[:, 0:64].rearrange("q (h e) -> q h e", h=2), AF.Copy)
                nc.scalar.activation(proj_sb[:, t, :], ps[:, 64:NPROJ], AF.Copy)

            # ---- phase C: nonlinearities + weights + indices ----
            nc.scalar.activation(proj_sb[:, :, 0:68], proj_sb[:, :, 0:68],
                                 AF.Sigmoid)
            nc.vector.tensor_scalar(out=proj_sb[:, :, 0:4],
                                    in0=proj_sb[:, :, 0:4],
                                    scalar1=0.25, scalar2=0.75,
                                    op0=A.max, op1=A.min)
            nc.scalar.activation(proj_sb[:, :, 68:100], proj_sb[:, :, 68:100],
                                 AF.Exp)

            shp = [128, NT, 16]
            for h in (0, 1):
                sx = proj_sb[:, :, 2 * h:2 * h + 1]        # [128,48,1]
                sy = proj_sb[:, :, 2 * h + 1:2 * h + 2]
                anc = proj_sb[:, :, 4 + 32 * h:4 + 32 * h + 32].rearrange(
                    "q t (p j) -> q t p j", j=2)
                ox, oy = anc[:, :, :, 0], anc[:, :, :, 1]
                att = proj_sb[:, :, 68 + 16 * h:68 + 16 * h + 16]
                cx, cy = cent_sb[:, 0:48], cent_sb[:, 48:96]

                axc = tpool.tile([128, NT], fp32, tag="axc")
                nc.vector.scalar_tensor_tensor(
                    out=axc[:], in0=sx[:, :, 0], scalar=-0.5, in1=cx,
                    op0=A.mult, op1=A.add)
                ayc = tpool.tile([128, NT], fp32, tag="ayc")
                nc.vector.scalar_tensor_tensor(
                    out=ayc[:], in0=sy[:, :, 0], scalar=-0.5, in1=cy,
                    op0=A.mult, op1=A.add)

                def floorpath(o_ap, s_ap, a_t, scale, tagp):
                    # returns (frac, floor) tiles [128, NT, 16]
                    tp = tpool.tile(shp, fp32, tag=f"tp{tagp}")
                    tr = tpool.tile(shp, fp32, tag=f"tr{tagp}")
                    tg = tpool.tile(shp, fp32, tag="tg", name=f"tg{tagp}")
                    nc.vector.tensor_tensor(
                        out=tp[:], in0=o_ap, in1=s_ap.to_broadcast(shp),
                        op=A.mult)
                    nc.vector.tensor_tensor(
                        out=tp[:], in0=tp[:],
                        in1=a_t[:][:, :, None].to_broadcast(shp), op=A.add)
                    nc.vector.tensor_scalar(out=tp[:], in0=tp[:],
                                            scalar1=0.0, scalar2=1.0,
                                            op0=A.max, op1=A.min)
                    nc.vector.tensor_scalar(out=tr[:], in0=tp[:],
                                            scalar1=scale, scalar2=TWO23,
                                            op0=A.mult, op1=A.add)
                    nc.vector.tensor_scalar(out=tr[:], in0=tr[:],
                                            scalar1=TWO23, scalar2=None,
                                            op0=A.subtract)
                    nc.vector.tensor_scalar(out=tp[:], in0=tp[:],
                                            scalar1=scale, scalar2=None,
                                            op0=A.mult)
                    nc.vector.tensor_tensor(out=tg[:], in0=tr[:], in1=tp[:],
                                            op=A.is_gt)
                    nc.vector.tensor_tensor(out=tr[:], in0=tr[:], in1=tg[:],
                                            op=A.subtract)     # floor
                    nc.vector.tensor_tensor(out=tp[:], in0=tp[:], in1=tr[:],
                                            op=A.subtract)     # frac
                    return tp, tr

                wx, x0 = floorpath(ox, sx, axc, float(W - 1), "x")
                wy, y0 = floorpath(oy, sy, ayc, float(H - 1), "y")

                rf = tpool.tile(shp, fp32, tag="tg", name="rf")
                nc.vector.scalar_tensor_tensor(
                    out=rf[:], in0=y0[:], scalar=float(W), in1=x0[:],
                    op0=A.mult, op1=A.add)
                # p-major table row: r' = (r % 128) * NT + r // 128
                qq = tpool.tile(shp, fp32, tag="qq", name="qq")
                gg = tpool.tile(shp, fp32, tag="gg2", name="gg2")
                nc.vector.tensor_scalar(out=qq[:], in0=rf[:],
                                        scalar1=1.0 / 128.0, scalar2=TWO23,
                                        op0=A.mult, op1=A.add)
                nc.vector.tensor_scalar(out=qq[:], in0=qq[:], scalar1=TWO23,
                                        scalar2=None, op0=A.subtract)
                nc.vector.tensor_scalar(out=gg[:], in0=rf[:],
                                        scalar1=1.0 / 128.0, scalar2=None,
                                        op0=A.mult)
                nc.vector.tensor_tensor(out=gg[:], in0=qq[:], in1=gg[:],
                                        op=A.is_gt)
                nc.vector.tensor_tensor(out=qq[:], in0=qq[:], in1=gg[:],
                                        op=A.subtract)          # r // 128
                nc.vector.scalar_tensor_tensor(
                    out=rf[:], in0=qq[:], scalar=-128.0, in1=rf[:],
                    op0=A.mult, op1=A.add)                      # r % 128
                nc.vector.scalar_tensor_tensor(
                    out=Rf[h][:].rearrange("q (t p) -> q t p", p=16),
                    in0=rf[:], scalar=float(NT), in1=qq[:],
                    op0=A.mult, op1=A.add)                      # r' 

                ex = tpool.tile(shp, fp32, tag="ex")
                nc.vector.tensor_scalar(out=ex[:], in0=wx[:], scalar1=-1.0,
                                        scalar2=1.0, op0=A.mult, op1=A.add)
                ey = tpool.tile(shp, fp32, tag="ey")
                nc.vector.tensor_scalar(out=ey[:], in0=wy[:], scalar1=-1.0,
                                        scalar2=1.0, op0=A.mult, op1=A.add)

                asum = tpool.tile([128, NT], fp32, tag="asum")
                nc.vector.tensor_reduce(out=asum[:], in_=att,
                                        axis=mybir.AxisListType.X, op=A.add)
                arec = tpool.tile([128, NT], fp32, tag="arec")
                nc.vector.reciprocal(arec[:], asum[:])
                an = tpool.tile(shp, fp32, tag="an")
                nc.vector.tensor_tensor(
                    out=an[:], in0=att,
                    in1=arec[:][:, :, None].to_broadcast(shp), op=A.mult)
                m0 = tpool.tile(shp, fp32, tag="m0")
                nc.vector.tensor_tensor(out=m0[:], in0=an[:], in1=ey[:],
                                        op=A.mult)
                nc.vector.tensor_tensor(out=an[:], in0=an[:], in1=wy[:],
                                        op=A.mult)             # an = m1
                c4v = C4[h][:].rearrange("q (t p s) -> q t p s", p=16, s=4)
                nc.vector.tensor_tensor(out=c4v[:, :, :, 0], in0=m0[:],
                                        in1=ex[:], op=A.mult)
                nc.vector.tensor_tensor(out=c4v[:, :, :, 1], in0=m0[:],
                                        in1=wx[:], op=A.mult)
                nc.vector.tensor_tensor(out=c4v[:, :, :, 2], in0=an[:],
                                        in1=ex[:], op=A.mult)
                nc.vector.tensor_tensor(out=c4v[:, :, :, 3], in0=an[:],
                                        in1=wx[:], op=A.mult)

                # fold R[q, (t,p)] -> IX[q%16, t*128 + p*8 + q//16] via PE:
                # R8[k, p, qh] = R[k, p] * (k//16 == qh);
                # IX_t[ql, p*8+qh] = sum_k F[k, ql] * R8[k, p, qh]
                for t in range(NT):
                    r8 = tpool.tile([128, 16, 8], fp32, tag="r8", name="r8",
                                    bufs=2)
                    nc.vector.tensor_tensor(
                        out=r8[:],
                        in0=Rf[h][:, t * 16:(t + 1) * 16][:, :, None]
                            .to_broadcast([128, 16, 8]),
                        in1=fold_sb[:, 16:24][:, None, :]
                            .to_broadcast([128, 16, 8]),
                        op=A.mult)
                    psI = psT.tile([32, 128], fp32, tag="pstX", name="psI")
                    nc.tensor.matmul(psI[0:16, :], fold_sb[:, 0:16],
                                     r8[:].rearrange("k p e -> k (p e)"),
                                     start=True, stop=True)
                    nc.scalar.activation(
                        IX[h][0:16, t * 128:(t + 1) * 128], psI[0:16, :],
                        AF.Copy)
                nc.sync.dma_start(IX[h][16:32, :], IX[h][0:16, :])
                nc.sync.dma_start(IX[h][32:64, :], IX[h][0:32, :])
                nc.sync.dma_start(IX[h][64:128, :], IX[h][0:64, :])

            # ---- phase D: build tables on PE (partition shifts), flat DMA ----
            for t in range(NT):
                for blk, sh in enumerate((1, W, W + 1)):
                    sa, sb_ = shmats[sh]
                    psh = psT.tile([128, 64], fp32, tag="psh")
                    nc.tensor.matmul(psh[:], sa[:], P_sb[:, t, :],
                                     start=True, stop=False)
                    nc.tensor.matmul(psh[:], sb_[:], P_sb[:, t + 1, :],
                                     start=False, stop=True)
                    nc.scalar.activation(
                        T_sb[:, :, t, (blk + 1) * 32:(blk + 2) * 32],
                        psh[:].rearrange("q (h e) -> q h e", h=2), AF.Copy)
            for h in (0, 1):
                nc.sync.dma_start(
                    T_dram[h][:, :].rearrange("(p t) e -> p (t e)", p=128),
                    T_sb[:, h, :, :])

            # ---- phase E: gather + combine + point-reduce (per head) ----
            qctr = 0
            for h in (0, 1):
                for t in range(NT):
                    G = wpool.tile([128, 16, 128], bf16, tag="G", bufs=8)
                    nc.gpsimd.dma_gather(
                        out_ap=G[:],
                        in_ap=T_dram[h][:, :],
                        idxs_ap=IX[h][:, t * 128:(t + 1) * 128],
                        num_idxs=2048,
                        num_idxs_reg=2048,
                        elem_size=128,
                        single_packet=False,
                        queue_num=qctr % 4,
                    )
                    qctr += 1
                    gv = G[:].rearrange("q b (s d) -> q b s d", s=4)
                    c4b = C4[h][:, t * 64:(t + 1) * 64].rearrange(
                        "q (b s) -> q b s", s=4)[:, :, :, None].to_broadcast(
                            [128, 16, 4, 32])
                    nc.vector.tensor_tensor(out=gv, in0=gv, in1=c4b, op=A.mult)
                    U = wpool.tile([128, 16, 32], bf16, tag="U", bufs=3)
                    V = wpool.tile([128, 16, 32], bf16, tag="V", bufs=3)
                    nc.vector.tensor_tensor(out=U[:], in0=G[:, :, 0:32],
                                            in1=G[:, :, 32:64], op=A.add)
                    nc.vector.tensor_tensor(out=V[:], in0=G[:, :, 64:96],
                                            in1=G[:, :, 96:128], op=A.add)
                    S1 = wpool.tile([128, 16, 32], bf16, tag="S1", bufs=3)
                    nc.vector.tensor_tensor(out=S1[:], in0=U[:], in1=V[:],
                                            op=A.add)
                    pst = psT.tile([32, 128], fp32, tag="pstX", name="pst")
                    for p in range(16):
                        nc.tensor.matmul(pst[:], S1[:, p, :], ident[:],
                                         start=(p == 0), stop=(p == 15))
                    nc.scalar.activation(HO[h * 32:(h + 1) * 32, t, :],
                                         pst[:], AF.Copy)

            # ---- phase F: 8-core AllToAll + out_proj + BN ----
            # core m sends its (batch, head-pair) ho slice for l-slice j to
            # core j; each core ends with ALL (b, h) channels for its L/8.
            for j in range(8):
                nc.sync.dma_start(
                    ho_bounce[j * 64:(j + 1) * 64, :].rearrange(
                        "r (t e) -> r t e", e=128),
                    HO[:, j * 6:(j + 1) * 6, :])
            nc.gpsimd.collective_compute(
                "AllToAll",
                A.bypass,
                replica_groups=[[0, 1, 2, 3, 4, 5, 6, 7]],
                ins=[ho_bounce.opt()],
                outs=[a2a_out.opt()],
            )
            # a2a_out rows: (bb, kc, p) -> channel kc*128+p of batch bb
            rhs_sb = ppool.tile([128, 4, LQ], bf16)
            nc.sync.dma_start(
                rhs_sb[:],
                a2a_out[:, :].rearrange("(bb kc p) n -> p (bb kc) n",
                                        bb=2, kc=2))
            out_sb = ppool.tile([128, 4, LQ], fp32, tag="bigb", name="out_sb",
                                padded_shape=[128, 4, NT * PCOLS // 4])
            for bb in range(2):
                for cc in range(2):
                    for l0, ln in ((0, 512), (512, 256)):
                        pso = psO.tile([128, 512], fp32)
                        for kc in range(2):
                            nc.tensor.matmul(
                                pso[:, 0:ln],
                                wot_sb[:, kc, cc * 128:(cc + 1) * 128],
                                rhs_sb[:, 2 * bb + kc, l0:l0 + ln],
                                start=(kc == 0), stop=(kc == 1))
                        nc.vector.tensor_scalar(
                            out=out_sb[:, 2 * bb + cc, l0:l0 + ln],
                            in0=pso[:, 0:ln],
                            scalar1=bnsc_sb[:, cc:cc + 1],
                            scalar2=bnbi_sb[:, cc:cc + 1],
                            op0=A.mult, op1=A.add)
            nc.sync.dma_start(
                out[:, :].rearrange("(q p) n -> p q n", q=4), out_sb[:])

    nc.finalize()
    return nc


def _prep_inputs(inputs):
    f = np.float32
    feat_sd = np.asarray(inputs['feat_sd'], dtype=f)
    w_size = np.asarray(inputs['w_size'], dtype=f)
    b_size = np.asarray(inputs['b_size'], dtype=f)
    w_anchor = np.asarray(inputs['w_anchor'], dtype=f)
    b_anchor = np.asarray(inputs['b_anchor'], dtype=f)
    w_value = np.asarray(inputs['w_value'], dtype=f)
    b_value = np.asarray(inputs['b_value'], dtype=f)
    w_att = np.asarray(inputs['w_att'], dtype=f)
    b_att = np.asarray(inputs['b_att'], dtype=f)
    w_out = np.asarray(inputs['w_out'], dtype=f)
    bn_gamma = np.asarray(inputs['bn_gamma'], dtype=f)
    bn_beta = np.asarray(inputs['bn_beta'], dtype=f)
    bn_mean = np.asarray(inputs['bn_mean'], dtype=f)
    bn_var = np.asarray(inputs['bn_var'], dtype=f)

    import ml_dtypes
    wot = np.ascontiguousarray(w_out.T).astype(ml_dtypes.bfloat16)
    scale = (bn_gamma / np.sqrt(bn_var + np.float32(1e-5))).astype(f)
    bias = (bn_beta - bn_mean * scale).astype(f)
    bnsc = np.ascontiguousarray(scale.reshape(2, 128).T)
    bnbi = np.ascontiguousarray(bias.reshape(2, 128).T)

    k = np.arange(128)
    foldm = np.zeros((128, 24), np.float32)
    foldm[k, k % 16] = 1.0
    foldm[k, 16 + k // 16] = 1.0

    l = np.arange(L).reshape(NT, 128)
    cx = ((l % W + 0.5).astype(f) / np.float32(W + EPS)).T
    cy = ((l // W + 0.5).astype(f) / np.float32(H + EPS)).T
    cent = np.ascontiguousarray(np.concatenate([cx, cy], axis=1), dtype=f)

    in_maps = []
    for m in range(8):
        b = m // 4
        h0 = 2 * (m % 4)
        h1 = h0 + 1
        wrows = np.concatenate([
            w_value[h0 * 32:(h0 + 1) * 32],
            w_value[h1 * 32:(h1 + 1) * 32],
            w_size[[2 * h0, 2 * h0 + 1, 2 * h1, 2 * h1 + 1]],
            w_anchor[h0 * 32:(h0 + 1) * 32],
            w_anchor[h1 * 32:(h1 + 1) * 32],
            w_att[h0 * 16:(h0 + 1) * 16],
            w_att[h1 * 16:(h1 + 1) * 16],
        ], axis=0)
        brows = np.concatenate([
            b_value[h0 * 32:(h0 + 1) * 32],
            b_value[h1 * 32:(h1 + 1) * 32],
            b_size[[2 * h0, 2 * h0 + 1, 2 * h1, 2 * h1 + 1]],
            b_anchor[h0 * 32:(h0 + 1) * 32],
            b_anchor[h1 * 32:(h1 + 1) * 32],
            b_att[h0 * 16:(h0 + 1) * 16],
            b_att[h1 * 16:(h1 + 1) * 16],
        ], axis=0)
        in_maps.append({
            "feat": np.ascontiguousarray(feat_sd[b].reshape(C, L)),
            "wproj": np.ascontiguousarray(wrows.T),
            "bproj": np.ascontiguousarray(brows.reshape(1, NPROJ)),
            "wot": wot,
            "bnsc": bnsc,
            "bnbi": bnbi,
            "cent": cent,
            "fold": foldm,
        })
    return in_maps


def _run(inputs, trace=False):
    from concourse.bass_utils import run_bass_kernel_spmd
    if "nc" not in _CACHE:
        _CACHE["nc"] = _build_nc()
    nc = _CACHE["nc"]
    in_maps = _prep_inputs(inputs)
    res = run_bass_kernel_spmd(nc, in_maps, core_ids=list(range(8)),
                               trace=trace)
    full = np.empty((B, C, L), np.float32)
    for m in range(8):
        o = res.results[m]["out"].reshape(2, C, LQ)
        for bb in range(2):
            full[bb][:, m * LQ:(m + 1) * LQ] = o[bb]
    return full.reshape(B, C, H, W), res.exec_time_ns


def kernel(**inputs):
    out, _ = _run(inputs, trace=False)
    return out
